# revision 23
# baseline (speedup 1.0000x reference)
"""CRF forward-algorithm (log partition) kernel for 8 Trainium2 NeuronCores.

Strategy: segment-spliced exp-space scan.

The reference recurrence  fv' = logsumexp_prev(fv + T) + feat  is, in exp
space, a linear matvec chain  v' = diag(e_t) @ M @ v  with M = exp(T) fixed.
We split the T=16384 steps into S=1024 segments of L=16 and run all segments
in parallel from a guess vector, batched 129 columns per core so the PE array
runs dense [128x128] x [128x129] matmuls (full utilization) instead of
matvecs.  Products of positive matrices contract exponentially toward rank-1
(Perron-Frobenius; measured contraction ~9x per step for this M), so the true
correction at each segment junction is a pure scalar kappa, measured exactly
by re-running only the first D=3 steps of each segment from the previous
segment's endpoint (phase 2, also fully parallel — logsumexp commutes with
additive constants).  alpha = lse(final) + sum(kappa).

Step 0 of every segment has a closed form (the init is uniform, or one-hot
for the true chain start), so the host folds  (M^T u) * e_t  into the first
emission tile and the device runs only L-1+D matmul steps.  Per-step
rescaling is folded into the emissions as a constant e^-8 (zero cost); all
bookkeeping scales are recovered analytically at the end.

Each core is fully independent (no collectives): core c owns segments
[c*128, c*128+128] (129 columns, one redundant boundary column so junction
sources are always core-local).  The host does the tiny O(S*N) final
assembly (kappa extraction + terminal logsumexp) in fp64.
"""

import numpy as np
import ml_dtypes

import concourse.bass as bass
import concourse.bacc as bacc
import concourse.mybir as mybir
import concourse.tile as tile

BF16_NP = ml_dtypes.bfloat16
FP8_NP = ml_dtypes.float8_e4m3
BF16 = mybir.dt.bfloat16
FP8 = mybir.dt.float8e4
F32 = mybir.dt.float32

SEQ_LEN = 16384
N_TAGS = 1024
START_IDX = 1022
STOP_IDX = 1023
NB = 8                 # 1024 tags = 8 blocks of 128 partitions
L = 16                 # segment length (steps)
D = 2                  # junction fixup depth (steps; contraction ~9x/step)
S = SEQ_LEN // L       # 1024 segments
NCORES = 8
BPC = S // NCORES      # 128 segments owned per core
NCOLS = BPC            # phase-1 columns (junction targets for the next
                       # core's first segment come from that core's own
                       # snap col 0 — the host splices across cores)
CSCALE = 8.0           # constant per-step rescale folded into emissions
ZB = 2                 # output-tag blocks computed in the final fixup step
                       # (kappa is a scalar per junction; 256 clean ratios
                       # pin its median, and blocks 0-1 avoid START/STOP)

_CACHE = {}


def _build_program():
    nc = bacc.Bacc("TRN2", target_bir_lowering=False, debug=False)
    # mt is the host-permuted partition-major SBUF image of the stationary
    # operand: mt[p, mb*1024 + kb*128 + c] = M[kb*128+p, mb*128+c], so DMAs
    # are plain 2D slices (two batched DMAs keep issue cost off the
    # critical path; fp8 weights hard-fault the PE in mixed-dtype matmuls,
    # so bf16 it stays).
    mt = nc.dram_tensor("mt", [128, NB * N_TAGS], BF16, kind="ExternalInput")
    e1 = nc.dram_tensor("e1", [L, 128, NB * NCOLS], BF16, kind="ExternalInput")
    e2 = nc.dram_tensor("e2", [D, 128, NB * BPC], BF16, kind="ExternalInput")
    snap = nc.dram_tensor("snap", [128, ZB * NCOLS], BF16, kind="ExternalOutput")
    yend = nc.dram_tensor("yend", [128, NB], BF16, kind="ExternalOutput")
    zout = nc.dram_tensor("zout", [128, ZB * BPC], BF16, kind="ExternalOutput")

    with tile.TileContext(nc) as tc:
        with (
            tc.tile_pool(name="mpool", bufs=1) as mpool,
            tc.tile_pool(name="vpool", bufs=3) as vpool,
            tc.tile_pool(name="epool", bufs=3) as epool,
            tc.tile_pool(name="zpool", bufs=1) as zpool,
            tc.tile_pool(name="spool", bufs=1) as spool,
            tc.tile_pool(name="ypool", bufs=1) as ypool,
            tc.tile_pool(name="pspool", bufs=1, space="PSUM") as pspool,
        ):
            # Stationary operand, mb-major sections: group mb's 8 contraction
            # tiles live at mt_sb[:, mb*1024 + kb*128 : +128].  Two batched
            # DMAs (issued on Sync) keep issue cost off the critical path.
            # Emission tiles: early steps ride the SP ring BEHIND the mt
            # chunks (FIFO keeps mt at full priority, no WAR coupling);
            # everything else goes through the Activation HWDGE queue.
            def load_e(row, ncols, eng=None):
                et = epool.tile([128, NB * ncols], BF16, tag="e")
                (eng or nc.scalar).dma_start(et[:], row)
                return et

            # Step 0 is folded into e1[0] on the host: it IS the state after
            # one step, laid out exactly like the v tiles the matmuls consume.
            # It leads the Activation ring so group 0 un-gates early.
            et0 = load_e(e1[0], NCOLS)
            v_aps = [et0[:, kb * NCOLS:(kb + 1) * NCOLS] for kb in range(NB)]

            # The 2MB stationary operand is the startup long pole; one HWDGE
            # ring streams ~210 GB/s while the core sustains ~420 GB/s, so
            # split the chunks across BOTH rings (SP + Activation).
            mt_sb = mpool.tile([128, NB * N_TAGS], BF16)
            for lo, hi, eng in (
                (0, 2, nc.sync), (2, 4, nc.sync),
                (4, 6, nc.scalar), (6, 8, nc.scalar),
            ):
                eng.dma_start(
                    mt_sb[:, lo * N_TAGS:hi * N_TAGS],
                    mt[:, lo * N_TAGS:hi * N_TAGS],
                )

            def step(v_aps, e_row, ncols, stage_out=None, zstage=None,
                     groups=NB, e_eng=None):
                et = load_e(e_row, ncols, e_eng)
                new_aps = []
                for mb in range(groups):
                    ps = pspool.tile([128, ncols], F32, tag=f"ps{mb}")
                    for kb in range(NB):
                        sec = mb * N_TAGS + kb * 128
                        nc.tensor.matmul(
                            ps[:],
                            mt_sb[:, sec:sec + 128],
                            v_aps[kb],
                            start=(kb == 0),
                            stop=(kb == NB - 1),
                        )
                    esl = et[:, mb * ncols:(mb + 1) * ncols]
                    if zstage is not None:
                        # final step: write straight into the contiguous
                        # staging tile; one DMA ships all 8 blocks.
                        nc.vector.tensor_mul(
                            zstage[:, mb * ncols:(mb + 1) * ncols], ps[:], esl
                        )
                    else:
                        nv = vpool.tile([128, ncols], BF16, tag=f"v{mb}")
                        nc.vector.tensor_mul(nv[:], ps[:], esl)
                        if stage_out is not None:
                            # copy on the idle GpSimd engine into a staging
                            # tile so the (late-completing) output DMA never
                            # holds a WAR on the live v slots.
                            stage, dram, cols = stage_out
                            if cols is None:
                                if mb < ZB:
                                    nc.gpsimd.tensor_copy(
                                        stage[:, mb * ncols:(mb + 1) * ncols],
                                        nv[:],
                                    )
                            else:
                                # single-column capture (final LSE source);
                                # DVE — GpSimd faults on 2-byte-wide APs
                                nc.vector.tensor_copy(
                                    stage[:, mb:mb + 1], nv[:, cols:cols + 1]
                                )
                        new_aps.append(nv[:])
                return new_aps

            # phase 1: steps 1..L-1 (step 0 host-folded)
            snap_stage = spool.tile([128, ZB * NCOLS], BF16)
            yend_stage = ypool.tile([128, NB], BF16)
            for s in range(1, L):
                stage_out = None
                if s + 1 == D:
                    stage_out = (snap_stage, snap, None)
                elif s + 1 == L:
                    stage_out = (yend_stage, yend, BPC - 1)
                v_aps = step(v_aps, e1[s], NCOLS, stage_out,
                             e_eng=nc.sync if s in (1, 2, 3) else None)

            # staged outputs ship once the startup-critical SP-ring window
            # has passed; the staging tiles are never overwritten, so these
            # DMAs carry no WAR against the live pipeline.
            nc.sync.dma_start(snap[:, :], snap_stage[:])
            nc.sync.dma_start(yend[:, :], yend_stage[:])

            # phase 2: D fixup steps from each segment's left-neighbor endpoint
            v_aps = [ap[:, 0:BPC] for ap in v_aps]
            for s in range(D):
                if s + 1 == D:
                    zstage = zpool.tile([128, ZB * BPC], BF16)
                    step(v_aps, e2[s], BPC, zstage=zstage, groups=ZB)
                    nc.sync.dma_start(zout[:, :], zstage[:])
                else:
                    v_aps = step(v_aps, e2[s], BPC)

    nc.compile()
    return nc


def _prepare_core_inputs(E, Mt_bf, w_unif, w_start):
    """Per-core input dicts. E: [T, N] f32 emissions exp(decoded - CSCALE)."""
    in_maps = []
    # partition-major SBUF image: mt[p, mb*1024 + kb*128 + c] = M[kb*128+p, mb*128+c]
    Mt_img = np.ascontiguousarray(
        Mt_bf.reshape(8, 128, 8, 128).transpose(1, 2, 0, 3).reshape(128, 8192)
    )
    steps1 = np.arange(L)
    steps2 = np.arange(D)
    for c in range(NCORES):
        segs1 = np.minimum(c * BPC + np.arange(NCOLS), S - 1)
        segs2 = np.minimum(c * BPC + 1 + np.arange(BPC), S - 1)
        t1 = segs1 * L  # [NCOLS]
        t2 = segs2 * L  # [BPC]
        # a1[s, col, tag] -> e1[s, p, mb*NCOLS + col]
        a1 = E[t1[None, :] + steps1[:, None]].copy()   # [L, NCOLS, N] f32
        a1[0] *= w_unif[None, :]                       # closed-form step 0
        if c == 0:
            a1[0, 0] = E[0] * w_start
        a1 = a1.astype(BF16_NP).reshape(L, NCOLS, NB, 128)
        e1 = np.ascontiguousarray(a1.transpose(0, 3, 2, 1)).reshape(L, 128, NB * NCOLS)
        a2 = E[t2[None, :] + steps2[:, None]].astype(BF16_NP)  # [D, BPC, N]
        a2 = a2.reshape(D, BPC, NB, 128)
        e2 = np.ascontiguousarray(a2.transpose(0, 3, 2, 1)).reshape(D, 128, NB * BPC)
        in_maps.append({"mt": Mt_img, "e1": e1, "e2": e2})
    return in_maps


def _prepare_in_maps(decoded, transitions):
    decoded = np.asarray(decoded, dtype=np.float32)
    transitions = np.asarray(transitions, dtype=np.float32)
    M64 = np.exp(transitions.astype(np.float64)).T          # [prev, next]
    Mt_bf = M64.astype(BF16_NP)
    w_unif = (M64.sum(axis=0) / N_TAGS).astype(np.float32)  # (M^T u)[next]
    w_start = M64[START_IDX].astype(np.float32)             # (M^T e_start)[next]
    E = np.exp(decoded - np.float32(CSCALE))                # [T, N] f32
    return _prepare_core_inputs(E, Mt_bf, w_unif, w_start)


def _assemble(transitions, results):
    """Host-side kappa extraction + terminal logsumexp (tiny, fp64)."""
    NT2 = ZB * 128
    # snap_all[:, s] = segment s state at depth D (tag blocks 0..ZB-1)
    snap_all = np.concatenate(
        [results[c]["snap"].astype(np.float64)
         .reshape(128, ZB, NCOLS).transpose(1, 0, 2).reshape(NT2, NCOLS)
         for c in range(NCORES)], axis=1)                    # [NT2, S]
    # z_all[:, j] from core c covers junction segments c*BPC+1 .. c*BPC+BPC
    z_all = np.concatenate(
        [results[c]["zout"].astype(np.float64)
         .reshape(128, ZB, BPC).transpose(1, 0, 2).reshape(NT2, BPC)
         for c in range(NCORES)], axis=1)                    # [NT2, S], seg j+1
    z = z_all[:, :S - 1]          # junctions for segments 1..S-1
    sn = snap_all[:, 1:S]
    valid = (z > 0) & (sn > 0)
    with np.errstate(divide="ignore", invalid="ignore"):
        dlt = np.where(valid, np.log(z) - np.log(sn), np.nan)
    kappa_sum = float(np.nansum(np.nanmedian(dlt, axis=0)))
    max_spread = float(np.nanmax(np.nanmax(dlt, axis=0) - np.nanmin(dlt, axis=0)))

    yraw = results[NCORES - 1]["yend"].astype(np.float64)    # [128, NB]
    y_last = yraw.T.reshape(N_TAGS)                          # tag = mb*128+p
    with np.errstate(divide="ignore"):
        logx = np.log(y_last) + kappa_sum + CSCALE * SEQ_LEN
    term = logx + transitions[STOP_IDX].astype(np.float64)
    term = term[np.isfinite(term)]
    mx = term.max()
    alpha = mx + np.log(np.exp(term - mx).sum())
    return alpha, max_spread


def kernel(decoded, transitions, raw_outputs=None, outputs=None, _backend="hw"):
    transitions = np.asarray(transitions, dtype=np.float32)
    in_maps = _prepare_in_maps(decoded, transitions)

    if "nc" not in _CACHE:
        _CACHE["nc"] = _build_program()
    nc = _CACHE["nc"]

    if _backend == "sim":
        from concourse.bass_interp import CoreSim
        results = []
        for c in range(NCORES):
            sim = CoreSim(nc, trace=False)
            for k, v in in_maps[c].items():
                sim.tensor(k)[:] = v
            sim.simulate()
            results.append({k: np.array(sim.tensor(k)) for k in ("snap", "yend", "zout")})
    else:
        from concourse.bass_utils import run_bass_kernel_spmd
        res = run_bass_kernel_spmd(nc, in_maps, list(range(NCORES)))
        results = res.results

    alpha, max_spread = _assemble(transitions, results)
    if max_spread > 0.2:
        import sys
        print(f"kernel: WARNING junction spread {max_spread:.3e}", file=sys.stderr)
    return np.float32(alpha)
